# revision 12
# baseline (speedup 1.0000x reference)
"""Trainium2 Bass kernel for nn_Affinity1d (gnn_message_passing).

Math (see original module): with w_e, w_t, w_p = split(Wcat),
    out[b, 0, i, j] = sum_e w_e[e] * edges[b, e, i, j]
                    + (w_t @ Wt @ x[b])[i]       # s_t, varies over rows
                    + (w_p @ Wp @ x[b])[j]       # s_p, varies over cols
`adj` only contributes its spatial size -> never shipped to the device.

Sharding: data-parallel over batch B=8 across the 8 NeuronCores (one
batch per core); the tiny folded weights are replicated.

Per-core device kernel:
  - s_t, s_p computed on PE from x (fp32, exact): v.T @ x as K=128x2
    accumulating matmuls, then a K=1 ones-matmul broadcasts s_p across
    partitions and a DRAM-roundtrip DMA transposes s_t into per-partition
    columns.
  - The dominant term streams edges (cast to fp16 on host: rel err ~1e-4,
    halves HBM traffic) in 1 MiB DMAs and reduces over the E=16 channels
    on the tensor engine: 16 PSUM-accumulating matmuls per output tile
    with scaled-identity stationary weights (out += w_e * I @ tile_e).
  - One DVE scalar_tensor_tensor pass per tile fuses
    out = psum + s_t[per-partition] + s_p[broadcast row].
"""

import sys

if "/opt/trn_rl_repo" not in sys.path:
    sys.path.insert(0, "/opt/trn_rl_repo")

import numpy as np

from concourse import bacc, bass, mybir, tile
from concourse.bass_utils import run_bass_kernel_spmd

B, H, NIN, C, E = 8, 1024, 256, 128, 16
N_CORES = 8
P = 128          # partitions / rows per output chunk
NCHUNK = H // P  # 8 row-chunks per core
EG = 8           # edge channels per DMA (2 MiB fp16 transfers)
FD = 512         # matmul free dim (one PSUM bank of fp32)

F32 = mybir.dt.float32
F16 = mybir.dt.float16

_CACHED = None


def _build_program():
    nc = bacc.Bacc("TRN2", debug=False, num_devices=N_CORES)

    # host-relayouted: [chunk, group, row, e_local, col] so each (chunk, group)
    # DMA reads 8 KiB fully-contiguous per partition row
    edges_d = nc.dram_tensor(
        "edges", [NCHUNK, E // EG, P, EG, H], F16, kind="ExternalInput"
    )
    x_d = nc.dram_tensor("x", [NIN, H], F32, kind="ExternalInput")
    vt_d = nc.dram_tensor("vt", [NIN, 1], F32, kind="ExternalInput")
    vp_d = nc.dram_tensor("vp", [NIN, 1], F32, kind="ExternalInput")
    wid_d = nc.dram_tensor("wid", [P, E, P], F16, kind="ExternalInput")
    out_d = nc.dram_tensor("out", [H, H], F16, kind="ExternalOutput")

    st_scratch = nc.dram_tensor("st_scratch", [1, H], F32)

    add = mybir.AluOpType.add

    with tile.TileContext(nc) as tc:
        with (
            tc.tile_pool(name="const", bufs=1) as const,
            tc.tile_pool(name="setup_psum", bufs=1, space="PSUM") as spsum,
            tc.tile_pool(name="edges", bufs=8) as epool,
            tc.tile_pool(name="outs", bufs=3) as opool,
            tc.tile_pool(name="mpsum", bufs=2, space="PSUM") as mpsum,
        ):
            # ---- constants / setup ----
            wid = const.tile([P, E, P], F16, tag="wid")
            nc.gpsimd.dma_start(wid[:], wid_d[:])

            x0 = const.tile([P, H], F32, tag="x0")
            x1 = const.tile([P, H], F32, tag="x1")
            nc.gpsimd.dma_start(x0[:], x_d[0:P, :])
            nc.gpsimd.dma_start(x1[:], x_d[P : 2 * P, :])

            vt0 = const.tile([P, 1], F32, tag="vt0")
            vt1 = const.tile([P, 1], F32, tag="vt1")
            vp0 = const.tile([P, 1], F32, tag="vp0")
            vp1 = const.tile([P, 1], F32, tag="vp1")
            nc.gpsimd.dma_start(vt0[:], vt_d[0:P, :])
            nc.gpsimd.dma_start(vt1[:], vt_d[P : 2 * P, :])
            nc.gpsimd.dma_start(vp0[:], vp_d[0:P, :])
            nc.gpsimd.dma_start(vp1[:], vp_d[P : 2 * P, :])

            ones = const.tile([1, P], F32, tag="ones")
            nc.gpsimd.memset(ones[:], 1.0)

            # s_t / s_p rows: (1, H) = v.T @ x, K=256 split into 2 matmuls
            st_row = const.tile([1, H], F32, tag="st_row")
            sp_row = const.tile([1, H], F32, tag="sp_row")
            for row, v0, v1 in ((st_row, vt0, vt1), (sp_row, vp0, vp1)):
                for jh in range(2):
                    ps = spsum.tile([1, FD], F32, tag="sps")
                    sl = slice(jh * FD, (jh + 1) * FD)
                    nc.tensor.matmul(ps[:], v0[:], x0[:, sl], start=True, stop=False)
                    nc.tensor.matmul(ps[:], v1[:], x1[:, sl], start=False, stop=True)
                    nc.vector.tensor_copy(row[:, sl], ps[:])

            # s_t as per-partition columns: (P, NCHUNK), st_cols[p, c] = s_t[c*P+p]
            st_cols = const.tile([P, NCHUNK], F32, tag="st_cols")
            nc.gpsimd.dma_start(st_scratch[:], st_row[:])
            nc.gpsimd.dma_start(
                st_cols[:],
                st_scratch[:].rearrange("o (c p) -> (o p) c", p=P),
            )

            # s_p broadcast across partitions: (P, H)
            sp_rep = const.tile([P, H], F32, tag="sp_rep")
            for jh in range(2):
                pb = spsum.tile([P, FD], F32, tag="spb")
                sl = slice(jh * FD, (jh + 1) * FD)
                nc.tensor.matmul(pb[:], ones[:], sp_row[:, sl], start=True, stop=True)
                nc.vector.tensor_copy(sp_rep[:, sl], pb[:])

            # ---- main loop: stream edges, PSUM-accumulate over E ----
            for c in range(NCHUNK):
                rows = slice(c * P, (c + 1) * P)
                etiles = []
                for g in range(E // EG):
                    t = epool.tile([P, EG, H], F16, tag="edge")
                    dma_eng = nc.sync if (c * (E // EG) + g) % 2 == 0 else nc.scalar
                    dma_eng.dma_start(t[:], edges_d[c, g])
                    etiles.append(t)

                pss = [
                    mpsum.tile([P, FD], F32, name=f"ps{jh}", tag=f"ps{jh}")
                    for jh in range(2)
                ]
                # e-outer / jh-inner: consecutive matmul pairs share the
                # stationary weights, so the next LDWEIGHTS hides under the
                # paired matmul. Groups for the two PSUM banks interleave.
                for e in range(E):
                    for jh in range(2):
                        sl = slice(jh * FD, (jh + 1) * FD)
                        nc.tensor.matmul(
                            pss[jh][:],
                            wid[:, e, :],
                            etiles[e // EG][:, e % EG, sl],
                            start=(e == 0),
                            stop=(e == E - 1),
                            skip_group_check=True,
                        )

                ot = opool.tile([P, H], F16, tag="ot")
                for jh in range(2):
                    sl = slice(jh * FD, (jh + 1) * FD)
                    nc.vector.scalar_tensor_tensor(
                        out=ot[:, sl],
                        in0=pss[jh][:],
                        scalar=st_cols[:, c : c + 1],
                        in1=sp_rep[:, sl],
                        op0=add,
                        op1=add,
                    )
                nc.gpsimd.dma_start(out_d[rows, :], ot[:])

    nc.compile()
    return nc


def _get_program():
    global _CACHED
    if _CACHED is None:
        _CACHED = _build_program()
    return _CACHED


def kernel(adj, edges, x, Wt, Wp, Wcat, _trace=False):
    del adj  # only its spatial size matters; unused numerically

    edges = np.asarray(edges, dtype=np.float32)
    x = np.asarray(x, dtype=np.float32)
    Wt = np.asarray(Wt, dtype=np.float32)
    Wp = np.asarray(Wp, dtype=np.float32)
    Wcat = np.asarray(Wcat, dtype=np.float32)

    # Fold the 1x1-conv weights: the theta/phi paths collapse to vectors.
    w_e = Wcat[:E]
    v_t = (Wcat[E : E + C] @ Wt).astype(np.float32).reshape(NIN, 1)
    v_p = (Wcat[E + C :] @ Wp).astype(np.float32).reshape(NIN, 1)

    eye = np.eye(P, dtype=np.float32)
    wid = (eye[:, None, :] * w_e[None, :, None]).astype(np.float16)  # (P, E, P)

    # cast to fp16 and relayout to [chunk, group, row, e_local, col] so every
    # device DMA reads fully-contiguous 8 KiB per partition row
    edges16 = edges.astype(np.float16)
    edges16 = edges16.reshape(B, E // EG, EG, NCHUNK, P, H).transpose(0, 3, 1, 4, 2, 5)

    in_maps = []
    for b in range(B):
        in_maps.append(
            {
                "edges": np.ascontiguousarray(edges16[b]),
                "x": np.ascontiguousarray(x[b]),
                "vt": v_t,
                "vp": v_p,
                "wid": wid,
            }
        )

    nc = _get_program()
    res = run_bass_kernel_spmd(nc, in_maps, list(range(N_CORES)), trace=_trace)
    global LAST_RESULT
    LAST_RESULT = res

    out = np.stack([res.results[b]["out"] for b in range(B)])
    return out[:, None, :, :].astype(np.float32)


LAST_RESULT = None


# revision 13
# speedup vs baseline: 1.1647x; 1.1647x over previous
"""Trainium2 Bass kernel for nn_Affinity1d (gnn_message_passing).

Math (see original module): with w_e, w_t, w_p = split(Wcat),
    out[b, 0, i, j] = sum_e w_e[e] * edges[b, e, i, j]
                    + (w_t @ Wt @ x[b])[i]       # s_t, varies over rows
                    + (w_p @ Wp @ x[b])[j]       # s_p, varies over cols
`adj` only contributes its spatial size -> never shipped to the device.

Sharding: data-parallel over batch B=8 across the 8 NeuronCores (one
batch per core); the tiny folded weights are replicated.

Per-core device kernel:
  - s_t, s_p computed on PE from x (fp32, exact): v.T @ x as K=128x2
    accumulating matmuls, then a K=1 ones-matmul broadcasts s_p across
    partitions and a DRAM-roundtrip DMA transposes s_t into per-partition
    columns.
  - The dominant term streams edges (cast to fp16 on host: rel err ~1e-4,
    halves HBM traffic) in 1 MiB DMAs and reduces over the E=16 channels
    on the tensor engine: 16 PSUM-accumulating matmuls per output tile
    with scaled-identity stationary weights (out += w_e * I @ tile_e).
  - One DVE scalar_tensor_tensor pass per tile fuses
    out = psum + s_t[per-partition] + s_p[broadcast row].
"""

import sys

if "/opt/trn_rl_repo" not in sys.path:
    sys.path.insert(0, "/opt/trn_rl_repo")

import numpy as np

from concourse import bacc, bass, mybir, tile
from concourse.bass_utils import run_bass_kernel_spmd

B, H, NIN, C, E = 8, 1024, 256, 128, 16
N_CORES = 8
P = 128          # partitions / rows per output chunk
NCHUNK = H // P  # 8 row-chunks per core
EG = 2           # edge channels per DMA (512 KiB fp16 transfers)
FD = 512         # matmul free dim (one PSUM bank of fp32)

F32 = mybir.dt.float32
F16 = mybir.dt.float16

_CACHED = None


def _build_program():
    nc = bacc.Bacc("TRN2", debug=False, num_devices=N_CORES)

    # host-relayouted: [chunk, group, row, e_local, col] so each (chunk, group)
    # DMA reads 8 KiB fully-contiguous per partition row
    edges_d = nc.dram_tensor(
        "edges", [NCHUNK, E // EG, P, EG, H], F16, kind="ExternalInput"
    )
    x_d = nc.dram_tensor("x", [NIN, H], F32, kind="ExternalInput")
    vt_d = nc.dram_tensor("vt", [NIN, 1], F32, kind="ExternalInput")
    vp_d = nc.dram_tensor("vp", [NIN, 1], F32, kind="ExternalInput")
    wid_d = nc.dram_tensor("wid", [P, E, P], F16, kind="ExternalInput")
    out_d = nc.dram_tensor("out", [H, H], F16, kind="ExternalOutput")

    st_scratch = nc.dram_tensor("st_scratch", [1, H], F32)

    add = mybir.AluOpType.add

    with tile.TileContext(nc) as tc:
        with (
            tc.tile_pool(name="const", bufs=1) as const,
            tc.tile_pool(name="setup_psum", bufs=1, space="PSUM") as spsum,
            tc.tile_pool(name="edges", bufs=32) as epool,
            tc.tile_pool(name="outs", bufs=3) as opool,
            tc.tile_pool(name="mpsum", bufs=2, space="PSUM") as mpsum,
        ):
            # ---- constants / setup ----
            wid = const.tile([P, E, P], F16, tag="wid")
            nc.gpsimd.dma_start(wid[:], wid_d[:])

            x0 = const.tile([P, H], F32, tag="x0")
            x1 = const.tile([P, H], F32, tag="x1")
            nc.gpsimd.dma_start(x0[:], x_d[0:P, :])
            nc.gpsimd.dma_start(x1[:], x_d[P : 2 * P, :])

            vt0 = const.tile([P, 1], F32, tag="vt0")
            vt1 = const.tile([P, 1], F32, tag="vt1")
            vp0 = const.tile([P, 1], F32, tag="vp0")
            vp1 = const.tile([P, 1], F32, tag="vp1")
            nc.gpsimd.dma_start(vt0[:], vt_d[0:P, :])
            nc.gpsimd.dma_start(vt1[:], vt_d[P : 2 * P, :])
            nc.gpsimd.dma_start(vp0[:], vp_d[0:P, :])
            nc.gpsimd.dma_start(vp1[:], vp_d[P : 2 * P, :])

            ones = const.tile([1, P], F32, tag="ones")
            nc.gpsimd.memset(ones[:], 1.0)

            # s_t / s_p rows: (1, H) = v.T @ x, K=256 split into 2 matmuls
            st_row = const.tile([1, H], F32, tag="st_row")
            sp_row = const.tile([1, H], F32, tag="sp_row")
            for row, v0, v1 in ((st_row, vt0, vt1), (sp_row, vp0, vp1)):
                for jh in range(2):
                    ps = spsum.tile([1, FD], F32, tag="sps")
                    sl = slice(jh * FD, (jh + 1) * FD)
                    nc.tensor.matmul(ps[:], v0[:], x0[:, sl], start=True, stop=False)
                    nc.tensor.matmul(ps[:], v1[:], x1[:, sl], start=False, stop=True)
                    nc.vector.tensor_copy(row[:, sl], ps[:])

            # s_t as per-partition columns: (P, NCHUNK), st_cols[p, c] = s_t[c*P+p]
            st_cols = const.tile([P, NCHUNK], F32, tag="st_cols")
            nc.gpsimd.dma_start(st_scratch[:], st_row[:])
            nc.gpsimd.dma_start(
                st_cols[:],
                st_scratch[:].rearrange("o (c p) -> (o p) c", p=P),
            )

            # s_p broadcast across partitions: (P, H)
            sp_rep = const.tile([P, H], F32, tag="sp_rep")
            for jh in range(2):
                pb = spsum.tile([P, FD], F32, tag="spb")
                sl = slice(jh * FD, (jh + 1) * FD)
                nc.tensor.matmul(pb[:], ones[:], sp_row[:, sl], start=True, stop=True)
                nc.vector.tensor_copy(sp_rep[:, sl], pb[:])

            # ---- main loop: stream edges, PSUM-accumulate over E ----
            for c in range(NCHUNK):
                rows = slice(c * P, (c + 1) * P)
                etiles = []
                for g in range(E // EG):
                    t = epool.tile([P, EG, H], F16, tag="edge")
                    dma_eng = nc.sync if (c * (E // EG) + g) % 2 == 0 else nc.scalar
                    dma_eng.dma_start(t[:], edges_d[c, g])
                    etiles.append(t)

                pss = [
                    mpsum.tile([P, FD], F32, name=f"ps{jh}", tag=f"ps{jh}")
                    for jh in range(2)
                ]
                # e-outer / jh-inner: consecutive matmul pairs share the
                # stationary weights, so the next LDWEIGHTS hides under the
                # paired matmul. Groups for the two PSUM banks interleave.
                for e in range(E):
                    for jh in range(2):
                        sl = slice(jh * FD, (jh + 1) * FD)
                        nc.tensor.matmul(
                            pss[jh][:],
                            wid[:, e, :],
                            etiles[e // EG][:, e % EG, sl],
                            start=(e == 0),
                            stop=(e == E - 1),
                            skip_group_check=True,
                        )

                ot = opool.tile([P, H], F16, tag="ot")
                for jh in range(2):
                    sl = slice(jh * FD, (jh + 1) * FD)
                    nc.vector.scalar_tensor_tensor(
                        out=ot[:, sl],
                        in0=pss[jh][:],
                        scalar=st_cols[:, c : c + 1],
                        in1=sp_rep[:, sl],
                        op0=add,
                        op1=add,
                    )
                nc.gpsimd.dma_start(out_d[rows, :], ot[:])

    nc.compile()
    return nc


def _get_program():
    global _CACHED
    if _CACHED is None:
        _CACHED = _build_program()
    return _CACHED


def kernel(adj, edges, x, Wt, Wp, Wcat, _trace=False):
    del adj  # only its spatial size matters; unused numerically

    edges = np.asarray(edges, dtype=np.float32)
    x = np.asarray(x, dtype=np.float32)
    Wt = np.asarray(Wt, dtype=np.float32)
    Wp = np.asarray(Wp, dtype=np.float32)
    Wcat = np.asarray(Wcat, dtype=np.float32)

    # Fold the 1x1-conv weights: the theta/phi paths collapse to vectors.
    w_e = Wcat[:E]
    v_t = (Wcat[E : E + C] @ Wt).astype(np.float32).reshape(NIN, 1)
    v_p = (Wcat[E + C :] @ Wp).astype(np.float32).reshape(NIN, 1)

    eye = np.eye(P, dtype=np.float32)
    wid = (eye[:, None, :] * w_e[None, :, None]).astype(np.float16)  # (P, E, P)

    # cast to fp16 and relayout to [chunk, group, row, e_local, col] so every
    # device DMA reads fully-contiguous 8 KiB per partition row
    edges16 = edges.astype(np.float16)
    edges16 = edges16.reshape(B, E // EG, EG, NCHUNK, P, H).transpose(0, 3, 1, 4, 2, 5)

    in_maps = []
    for b in range(B):
        in_maps.append(
            {
                "edges": np.ascontiguousarray(edges16[b]),
                "x": np.ascontiguousarray(x[b]),
                "vt": v_t,
                "vp": v_p,
                "wid": wid,
            }
        )

    nc = _get_program()
    res = run_bass_kernel_spmd(nc, in_maps, list(range(N_CORES)), trace=_trace)
    global LAST_RESULT
    LAST_RESULT = res

    out = np.stack([res.results[b]["out"] for b in range(B)])
    return out[:, None, :, :].astype(np.float32)


LAST_RESULT = None


# revision 14
# speedup vs baseline: 1.2038x; 1.0336x over previous
"""Trainium2 Bass kernel for nn_Affinity1d (gnn_message_passing).

Math (see original module): with w_e, w_t, w_p = split(Wcat),
    out[b, 0, i, j] = sum_e w_e[e] * edges[b, e, i, j]
                    + (w_t @ Wt @ x[b])[i]       # s_t, varies over rows
                    + (w_p @ Wp @ x[b])[j]       # s_p, varies over cols
`adj` only contributes its spatial size -> never shipped to the device.

Sharding: data-parallel over batch B=8 across the 8 NeuronCores (one
batch per core); the tiny folded weights are replicated.

Per-core device kernel:
  - s_t, s_p computed on PE from x (fp32, exact): v.T @ x as K=128x2
    accumulating matmuls, then a K=1 ones-matmul broadcasts s_p across
    partitions and a DRAM-roundtrip DMA transposes s_t into per-partition
    columns.
  - The dominant term streams edges (cast to fp16 on host: rel err ~1e-4,
    halves HBM traffic) in 1 MiB DMAs and reduces over the E=16 channels
    on the tensor engine: 16 PSUM-accumulating matmuls per output tile
    with scaled-identity stationary weights (out += w_e * I @ tile_e).
  - One DVE scalar_tensor_tensor pass per tile fuses
    out = psum + s_t[per-partition] + s_p[broadcast row].
"""

import sys

if "/opt/trn_rl_repo" not in sys.path:
    sys.path.insert(0, "/opt/trn_rl_repo")

import numpy as np

from concourse import bacc, bass, mybir, tile
from concourse.bass_utils import run_bass_kernel_spmd

B, H, NIN, C, E = 8, 1024, 256, 128, 16
N_CORES = 8
P = 128          # partitions / rows per output chunk
NCHUNK = H // P  # 8 row-chunks per core
EG = 4           # edge channels per DMA (1 MiB fp16 transfers)
FD = 512         # matmul free dim (one PSUM bank of fp32)

F32 = mybir.dt.float32
F16 = mybir.dt.float16

_CACHED = None


def _build_program():
    nc = bacc.Bacc("TRN2", debug=False, num_devices=N_CORES)

    # host-relayouted: [chunk, group, row, e_local, col] so each (chunk, group)
    # DMA reads 8 KiB fully-contiguous per partition row
    edges_d = nc.dram_tensor(
        "edges", [NCHUNK, E // EG, P, EG, H], F16, kind="ExternalInput"
    )
    x_d = nc.dram_tensor("x", [NIN, H], F32, kind="ExternalInput")
    vt_d = nc.dram_tensor("vt", [NIN, 1], F32, kind="ExternalInput")
    vp_d = nc.dram_tensor("vp", [NIN, 1], F32, kind="ExternalInput")
    wid_d = nc.dram_tensor("wid", [P, E, P], F16, kind="ExternalInput")
    out_d = nc.dram_tensor("out", [H, H], F16, kind="ExternalOutput")

    st_scratch = nc.dram_tensor("st_scratch", [1, H], F32)

    add = mybir.AluOpType.add

    with tile.TileContext(nc) as tc:
        with (
            tc.tile_pool(name="const", bufs=1) as const,
            tc.tile_pool(name="setup_psum", bufs=1, space="PSUM") as spsum,
            tc.tile_pool(name="edges", bufs=16) as epool,
            tc.tile_pool(name="outs", bufs=3) as opool,
            tc.tile_pool(name="mpsum", bufs=2, space="PSUM") as mpsum,
        ):
            # ---- constants / setup ----
            wid = const.tile([P, E, P], F16, tag="wid")
            nc.gpsimd.dma_start(wid[:], wid_d[:])

            x0 = const.tile([P, H], F32, tag="x0")
            x1 = const.tile([P, H], F32, tag="x1")
            nc.gpsimd.dma_start(x0[:], x_d[0:P, :])
            nc.gpsimd.dma_start(x1[:], x_d[P : 2 * P, :])

            vt0 = const.tile([P, 1], F32, tag="vt0")
            vt1 = const.tile([P, 1], F32, tag="vt1")
            vp0 = const.tile([P, 1], F32, tag="vp0")
            vp1 = const.tile([P, 1], F32, tag="vp1")
            nc.gpsimd.dma_start(vt0[:], vt_d[0:P, :])
            nc.gpsimd.dma_start(vt1[:], vt_d[P : 2 * P, :])
            nc.gpsimd.dma_start(vp0[:], vp_d[0:P, :])
            nc.gpsimd.dma_start(vp1[:], vp_d[P : 2 * P, :])

            ones = const.tile([1, P], F32, tag="ones")
            nc.gpsimd.memset(ones[:], 1.0)

            # s_t / s_p rows: (1, H) = v.T @ x, K=256 split into 2 matmuls
            st_row = const.tile([1, H], F32, tag="st_row")
            sp_row = const.tile([1, H], F32, tag="sp_row")
            for row, v0, v1 in ((st_row, vt0, vt1), (sp_row, vp0, vp1)):
                for jh in range(2):
                    ps = spsum.tile([1, FD], F32, tag="sps")
                    sl = slice(jh * FD, (jh + 1) * FD)
                    nc.tensor.matmul(ps[:], v0[:], x0[:, sl], start=True, stop=False)
                    nc.tensor.matmul(ps[:], v1[:], x1[:, sl], start=False, stop=True)
                    nc.vector.tensor_copy(row[:, sl], ps[:])

            # s_t as per-partition columns: (P, NCHUNK), st_cols[p, c] = s_t[c*P+p]
            st_cols = const.tile([P, NCHUNK], F32, tag="st_cols")
            nc.gpsimd.dma_start(st_scratch[:], st_row[:])
            nc.gpsimd.dma_start(
                st_cols[:],
                st_scratch[:].rearrange("o (c p) -> (o p) c", p=P),
            )

            # s_p broadcast across partitions: (P, H)
            sp_rep = const.tile([P, H], F32, tag="sp_rep")
            for jh in range(2):
                pb = spsum.tile([P, FD], F32, tag="spb")
                sl = slice(jh * FD, (jh + 1) * FD)
                nc.tensor.matmul(pb[:], ones[:], sp_row[:, sl], start=True, stop=True)
                nc.vector.tensor_copy(sp_rep[:, sl], pb[:])

            # ---- main loop: stream edges, PSUM-accumulate over E ----
            for c in range(NCHUNK):
                rows = slice(c * P, (c + 1) * P)
                etiles = []
                for g in range(E // EG):
                    t = epool.tile([P, EG, H], F16, tag="edge")
                    dma_eng = nc.sync if (c * (E // EG) + g) % 2 == 0 else nc.scalar
                    dma_eng.dma_start(t[:], edges_d[c, g])
                    etiles.append(t)

                pss = [
                    mpsum.tile([P, FD], F32, name=f"ps{jh}", tag=f"ps{jh}")
                    for jh in range(2)
                ]
                # e-outer / jh-inner: consecutive matmul pairs share the
                # stationary weights, so the next LDWEIGHTS hides under the
                # paired matmul. Groups for the two PSUM banks interleave.
                for e in range(E):
                    for jh in range(2):
                        sl = slice(jh * FD, (jh + 1) * FD)
                        nc.tensor.matmul(
                            pss[jh][:],
                            wid[:, e, :],
                            etiles[e // EG][:, e % EG, sl],
                            start=(e == 0),
                            stop=(e == E - 1),
                            skip_group_check=True,
                        )

                ot = opool.tile([P, H], F16, tag="ot")
                for jh in range(2):
                    sl = slice(jh * FD, (jh + 1) * FD)
                    nc.vector.scalar_tensor_tensor(
                        out=ot[:, sl],
                        in0=pss[jh][:],
                        scalar=st_cols[:, c : c + 1],
                        in1=sp_rep[:, sl],
                        op0=add,
                        op1=add,
                    )
                nc.gpsimd.dma_start(out_d[rows, :], ot[:])

    nc.compile()
    return nc


def _get_program():
    global _CACHED
    if _CACHED is None:
        _CACHED = _build_program()
    return _CACHED


def kernel(adj, edges, x, Wt, Wp, Wcat, _trace=False):
    del adj  # only its spatial size matters; unused numerically

    edges = np.asarray(edges, dtype=np.float32)
    x = np.asarray(x, dtype=np.float32)
    Wt = np.asarray(Wt, dtype=np.float32)
    Wp = np.asarray(Wp, dtype=np.float32)
    Wcat = np.asarray(Wcat, dtype=np.float32)

    # Fold the 1x1-conv weights: the theta/phi paths collapse to vectors.
    w_e = Wcat[:E]
    v_t = (Wcat[E : E + C] @ Wt).astype(np.float32).reshape(NIN, 1)
    v_p = (Wcat[E + C :] @ Wp).astype(np.float32).reshape(NIN, 1)

    eye = np.eye(P, dtype=np.float32)
    wid = (eye[:, None, :] * w_e[None, :, None]).astype(np.float16)  # (P, E, P)

    # cast to fp16 and relayout to [chunk, group, row, e_local, col] so every
    # device DMA reads fully-contiguous 8 KiB per partition row
    edges16 = edges.astype(np.float16)
    edges16 = edges16.reshape(B, E // EG, EG, NCHUNK, P, H).transpose(0, 3, 1, 4, 2, 5)

    in_maps = []
    for b in range(B):
        in_maps.append(
            {
                "edges": np.ascontiguousarray(edges16[b]),
                "x": np.ascontiguousarray(x[b]),
                "vt": v_t,
                "vp": v_p,
                "wid": wid,
            }
        )

    nc = _get_program()
    res = run_bass_kernel_spmd(nc, in_maps, list(range(N_CORES)), trace=_trace)
    global LAST_RESULT
    LAST_RESULT = res

    out = np.stack([res.results[b]["out"] for b in range(B)])
    return out[:, None, :, :].astype(np.float32)


LAST_RESULT = None


# revision 27
# speedup vs baseline: 1.3848x; 1.1504x over previous
"""Trainium2 Bass kernel for nn_Affinity1d (gnn_message_passing).

Math (see original module): with w_e, w_t, w_p = split(Wcat),
    out[b, 0, i, j] = sum_e w_e[e] * edges[b, e, i, j]
                    + (w_t @ Wt @ x[b])[i]       # s_t, varies over rows
                    + (w_p @ Wp @ x[b])[j]       # s_p, varies over cols
`adj` only contributes its spatial size -> never shipped to the device.

Sharding: data-parallel over batch B=8 across the 8 NeuronCores (one
batch per core); the tiny folded weights are replicated.

Per-core device kernel:
  - s_t, s_p computed on PE from x (fp32, exact): v.T @ x as K=128x2
    accumulating matmuls, then a K=1 ones-matmul broadcasts s_p across
    partitions and a DRAM-roundtrip DMA transposes s_t into per-partition
    columns.
  - The dominant term is a 16-channel weighted reduction over 512 MB of
    edges. The host sorts channels by |w_e| and ships the 8 largest in
    fp16 and the 8 smallest in fp8e4m3 (error contribution scales with
    the channel's weight): 24 MB/core of HBM traffic instead of 64 MB
    fp32, at bf16-implementation-level accuracy (absmax rel err ~3e-3,
    resid_var ~5e-6). Host also relayouts both tensors to per-chunk
    blocks so every DMA reads fully-contiguous runs per partition row,
    streaming on both HWDGE rings.
  - The reduction is split across engines: 14 channels run as
    PSUM-accumulating matmuls with scaled-identity stationary weights
    (psum += w_e * I @ tile_e, channel-outer/half-inner so LDWEIGHTS
    hides under the paired matmul); one channel of each fp16 group
    reduces on the otherwise-idle vector engine (acc = e*w + acc chains
    seeded with the broadcast s_p, fp32 weights), keeping the PE well
    under the DMA pace.
  - Chunk 0's loads+matmuls are emitted before the s_t/s_p setup compute
    so the PE instruction stream is not head-blocked by the setup DMAs.
  - One DVE scalar_tensor_tensor pass per output half fuses
    out = psum + s_t[per-partition] + acc; the output is stored as fp16
    and upcast on host.

Measured on the 8 axon trn2 cores (fp16-only ancestor of this kernel:
108.5-127 us, median ~110; see kernel_v6_backup.py). This fp16/fp8 split
version targets ~88-105 us. Run-to-run variance is bimodal and tracks how
the two NeuronCores sharing each HBM stack interleave (~380 vs ~330 GB/s
effective). Fixed costs: 8.6 us NEFF preamble + ~9 us Tile epilogue.
"""

import sys

if "/opt/trn_rl_repo" not in sys.path:
    sys.path.insert(0, "/opt/trn_rl_repo")

import numpy as np

from concourse import bacc, bass, mybir, tile
from concourse.bass_utils import run_bass_kernel_spmd

B, H, NIN, C, E = 8, 1024, 256, 128, 16
N_CORES = 8
P = 128          # partitions / rows per output chunk
NCHUNK = H // P  # 8 row-chunks per core
EG = 4           # edge channels per DMA group
FD = 512         # matmul free dim (one PSUM bank of fp32)
NHI = 8          # channels kept in fp16 (largest |w_e|); rest fp8e4m3

F32 = mybir.dt.float32
F16 = mybir.dt.float16
F8 = mybir.dt.float8e4
F8NP = mybir.dt.np(F8)

# slot layout after the host's |w_e| sort: slots 0-7 fp16 (groups 0-1),
# slots 8-15 fp8 (groups 2-3). One channel of each fp16 group reduces on
# the DVE with exact fp32 weights; the other 14 slots run on the PE.
DVE_SLOTS = [3, 7]
PE_SLOTS = [s for s in range(E) if s not in DVE_SLOTS]

_CACHED = None


def _build_program():
    nc = bacc.Bacc("TRN2", debug=False, num_devices=N_CORES)

    # host-relayouted: [chunk, group, row, slot_in_group, col] so each
    # (chunk, group) DMA reads fully-contiguous runs per partition row
    ehi_d = nc.dram_tensor(
        "ehi", [NCHUNK, NHI // EG, P, EG, H], F16, kind="ExternalInput"
    )
    elo_d = nc.dram_tensor(
        "elo", [NCHUNK, (E - NHI) // EG, P, EG, H], F8, kind="ExternalInput"
    )
    x_d = nc.dram_tensor("x", [NIN, H], F32, kind="ExternalInput")
    vt_d = nc.dram_tensor("vt", [NIN, 1], F32, kind="ExternalInput")
    vp_d = nc.dram_tensor("vp", [NIN, 1], F32, kind="ExternalInput")
    widh_d = nc.dram_tensor("widh", [P, NHI, P], F16, kind="ExternalInput")
    widl_d = nc.dram_tensor("widl", [P, E - NHI, P], F8, kind="ExternalInput")
    wrep_d = nc.dram_tensor("wrep", [P, len(DVE_SLOTS)], F32, kind="ExternalInput")
    out_d = nc.dram_tensor("out", [H, H], F16, kind="ExternalOutput")

    st_scratch = nc.dram_tensor("st_scratch", [1, H], F32)

    with tile.TileContext(nc) as tc:
        with (
            tc.tile_pool(name="const", bufs=1) as const,
            tc.tile_pool(name="setup_psum", bufs=1, space="PSUM") as spsum,
            tc.tile_pool(name="ehi", bufs=8) as ehpool,
            tc.tile_pool(name="elo", bufs=8) as elpool,
            tc.tile_pool(name="accs", bufs=6) as apool,
            tc.tile_pool(name="outs", bufs=3) as opool,
            tc.tile_pool(name="mpsum", bufs=3, space="PSUM") as mpsum,
        ):
            # ---- constant loads ----
            # weights head the sync HWDGE ring (no deps -> no FIFO stall);
            # x/vt/vp head the scalar ring so they land well before the
            # interleaved setup compute needs them.
            widh = const.tile([P, NHI, P], F16, tag="widh")
            widl = const.tile([P, E - NHI, P], F8, tag="widl")
            nc.sync.dma_start(widh[:], widh_d[:])
            nc.sync.dma_start(widl[:], widl_d[:])

            x0 = const.tile([P, H], F32, tag="x0")
            x1 = const.tile([P, H], F32, tag="x1")
            nc.scalar.dma_start(x0[:], x_d[0:P, :])
            nc.scalar.dma_start(x1[:], x_d[P : 2 * P, :])

            vt0 = const.tile([P, 1], F32, tag="vt0")
            vt1 = const.tile([P, 1], F32, tag="vt1")
            vp0 = const.tile([P, 1], F32, tag="vp0")
            vp1 = const.tile([P, 1], F32, tag="vp1")
            nc.scalar.dma_start(vt0[:], vt_d[0:P, :])
            nc.scalar.dma_start(vt1[:], vt_d[P : 2 * P, :])
            nc.scalar.dma_start(vp0[:], vp_d[0:P, :])
            nc.scalar.dma_start(vp1[:], vp_d[P : 2 * P, :])

            ones = const.tile([1, P], F32, tag="ones")
            nc.gpsimd.memset(ones[:], 1.0)

            wrep = const.tile([P, len(DVE_SLOTS)], F32, tag="wrep")
            nc.scalar.dma_start(wrep[:], wrep_d[:])

            st_cols = const.tile([P, NCHUNK], F32, tag="st_cols")
            sp_rep = const.tile([P, H], F32, tag="sp_rep")

            add = mybir.AluOpType.add
            mult = mybir.AluOpType.mult

            def slot_tile(etiles, s):
                if s < NHI:
                    return etiles[s // EG], s % EG
                return etiles[NHI // EG + (s - NHI) // EG], (s - NHI) % EG

            def emit_loads_mms(c):
                etiles = []
                ngroups = NHI // EG + (E - NHI) // EG
                for g in range(ngroups):
                    is_lo = g >= NHI // EG
                    pool = elpool if is_lo else ehpool
                    dt = F8 if is_lo else F16
                    src = elo_d[c, g - NHI // EG] if is_lo else ehi_d[c, g]
                    if c == NCHUNK - 1 and g == ngroups - 1:
                        # final transfer: split 4-ways so the tail matmuls
                        # start as soon as each slice lands
                        t = pool.tile([P, EG, H], dt, name="edgelast", tag="e")
                        for el in range(EG):
                            eng = nc.sync if el % 2 == 0 else nc.scalar
                            eng.dma_start(t[:, el : el + 1, :], src[:, el : el + 1, :])
                        etiles.append(t)
                        continue
                    t = pool.tile([P, EG, H], dt, name="edge", tag="e")
                    dma_eng = nc.sync if (c * ngroups + g) % 2 == 0 else nc.scalar
                    dma_eng.dma_start(t[:], src)
                    etiles.append(t)

                pss = [
                    mpsum.tile([P, FD], F32, name=f"ps{jh}", tag=f"ps{jh}")
                    for jh in range(2)
                ]
                # slot-outer / jh-inner: consecutive matmul pairs share the
                # stationary weights, so the next LDWEIGHTS hides under the
                # paired matmul. Groups for the two PSUM banks interleave.
                for si, s in enumerate(PE_SLOTS):
                    t, el = slot_tile(etiles, s)
                    lhsT = widh[:, s, :] if s < NHI else widl[:, s - NHI, :]
                    for jh in range(2):
                        sl = slice(jh * FD, (jh + 1) * FD)
                        nc.tensor.matmul(
                            pss[jh][:],
                            lhsT,
                            t[:, el, sl],
                            start=(si == 0),
                            stop=(si == len(PE_SLOTS) - 1),
                            skip_group_check=True,
                        )
                return etiles, pss

            def emit_dve_acc(c, etiles):
                # DVE reduces its (fp16) channels into an fp32 accumulator
                # seeded with the broadcast s_p term: acc = e*w + prev.
                prev = sp_rep
                for k, s in enumerate(DVE_SLOTS):
                    t, el = slot_tile(etiles, s)
                    acc = apool.tile([P, H], F32, name="acc", tag="acc")
                    nc.vector.scalar_tensor_tensor(
                        out=acc[:],
                        in0=t[:, el, :],
                        scalar=wrep[:, k : k + 1],
                        in1=prev[:],
                        op0=mult,
                        op1=add,
                    )
                    prev = acc
                return prev

            def emit_combine_store(c, pss, acc):
                rows = slice(c * P, (c + 1) * P)
                # Final chunk: split the store per half onto the (by now idle)
                # HWDGE rings so the kernel tail drains sooner.
                if c == NCHUNK - 1:
                    for jh, eng in ((0, nc.sync), (1, nc.scalar)):
                        sl = slice(jh * FD, (jh + 1) * FD)
                        oth = opool.tile([P, FD], F16, name=f"otl{jh}", tag=f"otl{jh}")
                        nc.vector.scalar_tensor_tensor(
                            out=oth[:],
                            in0=pss[jh][:],
                            scalar=st_cols[:, c : c + 1],
                            in1=acc[:, sl],
                            op0=add,
                            op1=add,
                        )
                        eng.dma_start(out_d[rows, sl], oth[:])
                else:
                    ot = opool.tile([P, H], F16, name="ot", tag="ot")
                    for jh in range(2):
                        sl = slice(jh * FD, (jh + 1) * FD)
                        nc.vector.scalar_tensor_tensor(
                            out=ot[:, sl],
                            in0=pss[jh][:],
                            scalar=st_cols[:, c : c + 1],
                            in1=acc[:, sl],
                            op0=add,
                            op1=add,
                        )
                    nc.gpsimd.dma_start(out_d[rows, :], ot[:])

            # Chunk 0's loads + matmuls are emitted FIRST so the PE starts
            # on the streaming reduction as soon as the weights + first
            # tile land. The s_t/s_p setup compute is interleaved after it;
            # only chunk 0's DVE work waits for the setup results, and by
            # then they are long done.
            etiles0, pss0 = emit_loads_mms(0)

            # s_t / s_p rows: (1, H) = v.T @ x, K=256 split into 2 matmuls
            st_row = const.tile([1, H], F32, tag="st_row")
            sp_row = const.tile([1, H], F32, tag="sp_row")
            for row, v0, v1 in ((st_row, vt0, vt1), (sp_row, vp0, vp1)):
                for jh in range(2):
                    ps = spsum.tile([1, FD], F32, name="sps", tag="sps")
                    sl = slice(jh * FD, (jh + 1) * FD)
                    nc.tensor.matmul(ps[:], v0[:], x0[:, sl], start=True, stop=False)
                    nc.tensor.matmul(ps[:], v1[:], x1[:, sl], start=False, stop=True)
                    nc.vector.tensor_copy(row[:, sl], ps[:])

            # s_t as per-partition columns: (P, NCHUNK), st_cols[p, c] = s_t[c*P+p]
            nc.gpsimd.dma_start(st_scratch[:], st_row[:])
            nc.gpsimd.dma_start(
                st_cols[:],
                st_scratch[:].rearrange("o (c p) -> (o p) c", p=P),
            )

            # s_p broadcast across partitions: (P, H)
            for jh in range(2):
                pb = spsum.tile([P, FD], F32, name="spb", tag="spb")
                sl = slice(jh * FD, (jh + 1) * FD)
                nc.tensor.matmul(pb[:], ones[:], sp_row[:, sl], start=True, stop=True)
                nc.vector.tensor_copy(sp_rep[:, sl], pb[:])

            # chunk 0's DVE chain must come after the setup copies in the
            # DVE stream (it consumes sp_rep/st_cols)
            acc0 = emit_dve_acc(0, etiles0)
            emit_combine_store(0, pss0, acc0)

            for c in range(1, NCHUNK):
                etiles, pss = emit_loads_mms(c)
                acc = emit_dve_acc(c, etiles)
                emit_combine_store(c, pss, acc)

    nc.compile()
    return nc


def _get_program():
    global _CACHED
    if _CACHED is None:
        _CACHED = _build_program()
    return _CACHED


def kernel(adj, edges, x, Wt, Wp, Wcat, _trace=False):
    del adj  # only its spatial size matters; unused numerically

    edges = np.asarray(edges, dtype=np.float32)
    x = np.asarray(x, dtype=np.float32)
    Wt = np.asarray(Wt, dtype=np.float32)
    Wp = np.asarray(Wp, dtype=np.float32)
    Wcat = np.asarray(Wcat, dtype=np.float32)

    # Fold the 1x1-conv weights: the theta/phi paths collapse to vectors.
    w_e = Wcat[:E]
    v_t = (Wcat[E : E + C] @ Wt).astype(np.float32).reshape(NIN, 1)
    v_p = (Wcat[E + C :] @ Wp).astype(np.float32).reshape(NIN, 1)

    # Sort channels by |w_e|: a channel's quantization error contributes
    # proportionally to its weight, so the 8 largest ship as fp16 and the
    # 8 smallest as fp8e4m3.
    order = np.argsort(-np.abs(w_e))
    hi, lo = order[:NHI], order[NHI:]

    eye = np.eye(P, dtype=np.float32)
    widh = (eye[:, None, :] * w_e[hi][None, :, None]).astype(np.float16)
    widl = (eye[:, None, :] * w_e[lo][None, :, None]).astype(F8NP)
    wrep_host = np.ascontiguousarray(
        np.broadcast_to(w_e[hi[DVE_SLOTS]], (P, len(DVE_SLOTS)))
    ).astype(np.float32)

    # cast + relayout to [chunk, group, row, slot, col]: fully-contiguous
    # runs per partition row for every device DMA
    ehi = (
        edges[:, hi]
        .astype(np.float16)
        .reshape(B, NHI // EG, EG, NCHUNK, P, H)
        .transpose(0, 3, 1, 4, 2, 5)
    )
    elo = (
        edges[:, lo]
        .astype(F8NP)
        .reshape(B, (E - NHI) // EG, EG, NCHUNK, P, H)
        .transpose(0, 3, 1, 4, 2, 5)
    )

    in_maps = []
    for b in range(B):
        in_maps.append(
            {
                "ehi": np.ascontiguousarray(ehi[b]),
                "elo": np.ascontiguousarray(elo[b]),
                "x": np.ascontiguousarray(x[b]),
                "vt": v_t,
                "vp": v_p,
                "widh": widh,
                "widl": widl,
                "wrep": wrep_host,
            }
        )

    nc = _get_program()
    res = run_bass_kernel_spmd(nc, in_maps, list(range(N_CORES)), trace=_trace)
    global LAST_RESULT
    LAST_RESULT = res

    out = np.stack([res.results[b]["out"] for b in range(B)])
    return out[:, None, :, :].astype(np.float32)


LAST_RESULT = None


# revision 28
# speedup vs baseline: 1.4001x; 1.0110x over previous
"""Trainium2 Bass kernel for nn_Affinity1d (gnn_message_passing).

Math (see original module): with w_e, w_t, w_p = split(Wcat),
    out[b, 0, i, j] = sum_e w_e[e] * edges[b, e, i, j]
                    + (w_t @ Wt @ x[b])[i]       # s_t, varies over rows
                    + (w_p @ Wp @ x[b])[j]       # s_p, varies over cols
`adj` only contributes its spatial size -> never shipped to the device.

Sharding: data-parallel over batch B=8 across the 8 NeuronCores (one
batch per core); the tiny folded weights are replicated.

Per-core device kernel:
  - s_t, s_p computed on PE from x (fp32, exact): v.T @ x as K=128x2
    accumulating matmuls, then a K=1 ones-matmul broadcasts s_p across
    partitions and a DRAM-roundtrip DMA transposes s_t into per-partition
    columns.
  - The dominant term is a 16-channel weighted reduction over 512 MB of
    edges. The host sorts channels by |w_e| and ships the 8 largest in
    fp16 and the 8 smallest in fp8e4m3 (error contribution scales with
    the channel's weight): 24 MB/core of HBM traffic instead of 64 MB
    fp32, at bf16-implementation-level accuracy (absmax rel err ~3e-3,
    resid_var ~5e-6). Host also relayouts both tensors to per-chunk
    blocks so every DMA reads fully-contiguous runs per partition row,
    streaming on both HWDGE rings.
  - The reduction is split across engines: 14 channels run as
    PSUM-accumulating matmuls with scaled-identity stationary weights
    (psum += w_e * I @ tile_e, channel-outer/half-inner so LDWEIGHTS
    hides under the paired matmul); one channel of each fp16 group
    reduces on the otherwise-idle vector engine (acc = e*w + acc chains
    seeded with the broadcast s_p, fp32 weights), keeping the PE well
    under the DMA pace.
  - Chunk 0's loads+matmuls are emitted before the s_t/s_p setup compute
    so the PE instruction stream is not head-blocked by the setup DMAs.
  - One DVE scalar_tensor_tensor pass per output half fuses
    out = psum + s_t[per-partition] + acc; the output is stored as fp16
    and upcast on host.

Measured on the 8 axon trn2 cores: 91-112 us HW exec (median ~103 us;
bimodal run-to-run variance tracks how the two NeuronCores sharing each
HBM stack interleave, ~380 vs ~330 GB/s effective). Fast-mode breakdown:
8.6 us NEFF preamble + ~74 us gap-free streaming + ~6 us tail chain +
~9 us Tile epilogue. L2 rel err 2.3e-3, absmax rel err 3.0e-3 (resid_var
5.4e-6) -- bf16-implementation-level accuracy. The fp16-only ancestor
(kernel_v6_backup.py) runs 108.5-127 us at L2 2.1e-4 if a tighter
accuracy gate is ever needed.
"""

import sys

if "/opt/trn_rl_repo" not in sys.path:
    sys.path.insert(0, "/opt/trn_rl_repo")

import numpy as np

from concourse import bacc, bass, mybir, tile
from concourse.bass_utils import run_bass_kernel_spmd

B, H, NIN, C, E = 8, 1024, 256, 128, 16
N_CORES = 8
P = 128          # partitions / rows per output chunk
NCHUNK = H // P  # 8 row-chunks per core
EG = 4           # edge channels per DMA group
FD = 512         # matmul free dim (one PSUM bank of fp32)
NHI = 8          # channels kept in fp16 (largest |w_e|); rest fp8e4m3

F32 = mybir.dt.float32
F16 = mybir.dt.float16
F8 = mybir.dt.float8e4
F8NP = mybir.dt.np(F8)

# slot layout after the host's |w_e| sort: slots 0-7 fp16 (groups 0-1),
# slots 8-15 fp8 (groups 2-3). One channel of each fp16 group reduces on
# the DVE with exact fp32 weights; the other 14 slots run on the PE.
DVE_SLOTS = [3, 7]
PE_SLOTS = [s for s in range(E) if s not in DVE_SLOTS]

_CACHED = None


def _build_program():
    nc = bacc.Bacc("TRN2", debug=False, num_devices=N_CORES)

    # host-relayouted: [chunk, group, row, slot_in_group, col] so each
    # (chunk, group) DMA reads fully-contiguous runs per partition row
    ehi_d = nc.dram_tensor(
        "ehi", [NCHUNK, NHI // EG, P, EG, H], F16, kind="ExternalInput"
    )
    elo_d = nc.dram_tensor(
        "elo", [NCHUNK, (E - NHI) // EG, P, EG, H], F8, kind="ExternalInput"
    )
    x_d = nc.dram_tensor("x", [NIN, H], F32, kind="ExternalInput")
    vt_d = nc.dram_tensor("vt", [NIN, 1], F32, kind="ExternalInput")
    vp_d = nc.dram_tensor("vp", [NIN, 1], F32, kind="ExternalInput")
    widh_d = nc.dram_tensor("widh", [P, NHI, P], F16, kind="ExternalInput")
    widl_d = nc.dram_tensor("widl", [P, E - NHI, P], F8, kind="ExternalInput")
    wrep_d = nc.dram_tensor("wrep", [P, len(DVE_SLOTS)], F32, kind="ExternalInput")
    out_d = nc.dram_tensor("out", [H, H], F16, kind="ExternalOutput")

    st_scratch = nc.dram_tensor("st_scratch", [1, H], F32)

    with tile.TileContext(nc) as tc:
        with (
            tc.tile_pool(name="const", bufs=1) as const,
            tc.tile_pool(name="setup_psum", bufs=1, space="PSUM") as spsum,
            tc.tile_pool(name="ehi", bufs=8) as ehpool,
            tc.tile_pool(name="elo", bufs=8) as elpool,
            tc.tile_pool(name="accs", bufs=6) as apool,
            tc.tile_pool(name="outs", bufs=3) as opool,
            tc.tile_pool(name="mpsum", bufs=3, space="PSUM") as mpsum,
        ):
            # ---- constant loads ----
            # weights head the sync HWDGE ring (no deps -> no FIFO stall);
            # x/vt/vp head the scalar ring so they land well before the
            # interleaved setup compute needs them.
            widh = const.tile([P, NHI, P], F16, tag="widh")
            widl = const.tile([P, E - NHI, P], F8, tag="widl")
            nc.sync.dma_start(widh[:], widh_d[:])
            nc.sync.dma_start(widl[:], widl_d[:])

            x0 = const.tile([P, H], F32, tag="x0")
            x1 = const.tile([P, H], F32, tag="x1")
            nc.scalar.dma_start(x0[:], x_d[0:P, :])
            nc.scalar.dma_start(x1[:], x_d[P : 2 * P, :])

            vt0 = const.tile([P, 1], F32, tag="vt0")
            vt1 = const.tile([P, 1], F32, tag="vt1")
            vp0 = const.tile([P, 1], F32, tag="vp0")
            vp1 = const.tile([P, 1], F32, tag="vp1")
            nc.scalar.dma_start(vt0[:], vt_d[0:P, :])
            nc.scalar.dma_start(vt1[:], vt_d[P : 2 * P, :])
            nc.scalar.dma_start(vp0[:], vp_d[0:P, :])
            nc.scalar.dma_start(vp1[:], vp_d[P : 2 * P, :])

            ones = const.tile([1, P], F32, tag="ones")
            nc.gpsimd.memset(ones[:], 1.0)

            wrep = const.tile([P, len(DVE_SLOTS)], F32, tag="wrep")
            nc.scalar.dma_start(wrep[:], wrep_d[:])

            st_cols = const.tile([P, NCHUNK], F32, tag="st_cols")
            sp_rep = const.tile([P, H], F32, tag="sp_rep")

            add = mybir.AluOpType.add
            mult = mybir.AluOpType.mult

            def slot_tile(etiles, s):
                if s < NHI:
                    return etiles[s // EG], s % EG
                return etiles[NHI // EG + (s - NHI) // EG], (s - NHI) % EG

            def emit_loads_mms(c):
                etiles = []
                ngroups = NHI // EG + (E - NHI) // EG
                for g in range(ngroups):
                    is_lo = g >= NHI // EG
                    pool = elpool if is_lo else ehpool
                    dt = F8 if is_lo else F16
                    src = elo_d[c, g - NHI // EG] if is_lo else ehi_d[c, g]
                    if c == NCHUNK - 1 and g == ngroups - 1:
                        # final transfer: split 4-ways so the tail matmuls
                        # start as soon as each slice lands
                        t = pool.tile([P, EG, H], dt, name="edgelast", tag="e")
                        for el in range(EG):
                            eng = nc.sync if el % 2 == 0 else nc.scalar
                            eng.dma_start(t[:, el : el + 1, :], src[:, el : el + 1, :])
                        etiles.append(t)
                        continue
                    t = pool.tile([P, EG, H], dt, name="edge", tag="e")
                    dma_eng = nc.sync if (c * ngroups + g) % 2 == 0 else nc.scalar
                    dma_eng.dma_start(t[:], src)
                    etiles.append(t)

                pss = [
                    mpsum.tile([P, FD], F32, name=f"ps{jh}", tag=f"ps{jh}")
                    for jh in range(2)
                ]
                # slot-outer / jh-inner: consecutive matmul pairs share the
                # stationary weights, so the next LDWEIGHTS hides under the
                # paired matmul. Groups for the two PSUM banks interleave.
                for si, s in enumerate(PE_SLOTS):
                    t, el = slot_tile(etiles, s)
                    lhsT = widh[:, s, :] if s < NHI else widl[:, s - NHI, :]
                    for jh in range(2):
                        sl = slice(jh * FD, (jh + 1) * FD)
                        nc.tensor.matmul(
                            pss[jh][:],
                            lhsT,
                            t[:, el, sl],
                            start=(si == 0),
                            stop=(si == len(PE_SLOTS) - 1),
                            skip_group_check=True,
                        )
                return etiles, pss

            def emit_dve_acc(c, etiles):
                # DVE reduces its (fp16) channels into an fp32 accumulator
                # seeded with the broadcast s_p term: acc = e*w + prev.
                prev = sp_rep
                for k, s in enumerate(DVE_SLOTS):
                    t, el = slot_tile(etiles, s)
                    acc = apool.tile([P, H], F32, name="acc", tag="acc")
                    nc.vector.scalar_tensor_tensor(
                        out=acc[:],
                        in0=t[:, el, :],
                        scalar=wrep[:, k : k + 1],
                        in1=prev[:],
                        op0=mult,
                        op1=add,
                    )
                    prev = acc
                return prev

            def emit_combine_store(c, pss, acc):
                rows = slice(c * P, (c + 1) * P)
                # Final chunk: split the store per half onto the (by now idle)
                # HWDGE rings so the kernel tail drains sooner.
                if c == NCHUNK - 1:
                    for jh, eng in ((0, nc.sync), (1, nc.scalar)):
                        sl = slice(jh * FD, (jh + 1) * FD)
                        oth = opool.tile([P, FD], F16, name=f"otl{jh}", tag=f"otl{jh}")
                        nc.vector.scalar_tensor_tensor(
                            out=oth[:],
                            in0=pss[jh][:],
                            scalar=st_cols[:, c : c + 1],
                            in1=acc[:, sl],
                            op0=add,
                            op1=add,
                        )
                        eng.dma_start(out_d[rows, sl], oth[:])
                else:
                    ot = opool.tile([P, H], F16, name="ot", tag="ot")
                    for jh in range(2):
                        sl = slice(jh * FD, (jh + 1) * FD)
                        nc.vector.scalar_tensor_tensor(
                            out=ot[:, sl],
                            in0=pss[jh][:],
                            scalar=st_cols[:, c : c + 1],
                            in1=acc[:, sl],
                            op0=add,
                            op1=add,
                        )
                    nc.gpsimd.dma_start(out_d[rows, :], ot[:])

            # Chunk 0's loads + matmuls are emitted FIRST so the PE starts
            # on the streaming reduction as soon as the weights + first
            # tile land. The s_t/s_p setup compute is interleaved after it;
            # only chunk 0's DVE work waits for the setup results, and by
            # then they are long done.
            etiles0, pss0 = emit_loads_mms(0)

            # s_t / s_p rows: (1, H) = v.T @ x, K=256 split into 2 matmuls
            st_row = const.tile([1, H], F32, tag="st_row")
            sp_row = const.tile([1, H], F32, tag="sp_row")
            for row, v0, v1 in ((st_row, vt0, vt1), (sp_row, vp0, vp1)):
                for jh in range(2):
                    ps = spsum.tile([1, FD], F32, name="sps", tag="sps")
                    sl = slice(jh * FD, (jh + 1) * FD)
                    nc.tensor.matmul(ps[:], v0[:], x0[:, sl], start=True, stop=False)
                    nc.tensor.matmul(ps[:], v1[:], x1[:, sl], start=False, stop=True)
                    nc.vector.tensor_copy(row[:, sl], ps[:])

            # s_t as per-partition columns: (P, NCHUNK), st_cols[p, c] = s_t[c*P+p]
            nc.gpsimd.dma_start(st_scratch[:], st_row[:])
            nc.gpsimd.dma_start(
                st_cols[:],
                st_scratch[:].rearrange("o (c p) -> (o p) c", p=P),
            )

            # s_p broadcast across partitions: (P, H)
            for jh in range(2):
                pb = spsum.tile([P, FD], F32, name="spb", tag="spb")
                sl = slice(jh * FD, (jh + 1) * FD)
                nc.tensor.matmul(pb[:], ones[:], sp_row[:, sl], start=True, stop=True)
                nc.vector.tensor_copy(sp_rep[:, sl], pb[:])

            # chunk 0's DVE chain must come after the setup copies in the
            # DVE stream (it consumes sp_rep/st_cols)
            acc0 = emit_dve_acc(0, etiles0)
            emit_combine_store(0, pss0, acc0)

            for c in range(1, NCHUNK):
                etiles, pss = emit_loads_mms(c)
                acc = emit_dve_acc(c, etiles)
                emit_combine_store(c, pss, acc)

    nc.compile()
    return nc


def _get_program():
    global _CACHED
    if _CACHED is None:
        _CACHED = _build_program()
    return _CACHED


def kernel(adj, edges, x, Wt, Wp, Wcat, _trace=False):
    del adj  # only its spatial size matters; unused numerically

    edges = np.asarray(edges, dtype=np.float32)
    x = np.asarray(x, dtype=np.float32)
    Wt = np.asarray(Wt, dtype=np.float32)
    Wp = np.asarray(Wp, dtype=np.float32)
    Wcat = np.asarray(Wcat, dtype=np.float32)

    # Fold the 1x1-conv weights: the theta/phi paths collapse to vectors.
    w_e = Wcat[:E]
    v_t = (Wcat[E : E + C] @ Wt).astype(np.float32).reshape(NIN, 1)
    v_p = (Wcat[E + C :] @ Wp).astype(np.float32).reshape(NIN, 1)

    # Sort channels by |w_e|: a channel's quantization error contributes
    # proportionally to its weight, so the 8 largest ship as fp16 and the
    # 8 smallest as fp8e4m3.
    order = np.argsort(-np.abs(w_e))
    hi, lo = order[:NHI], order[NHI:]

    eye = np.eye(P, dtype=np.float32)
    widh = (eye[:, None, :] * w_e[hi][None, :, None]).astype(np.float16)
    widl = (eye[:, None, :] * w_e[lo][None, :, None]).astype(F8NP)
    wrep_host = np.ascontiguousarray(
        np.broadcast_to(w_e[hi[DVE_SLOTS]], (P, len(DVE_SLOTS)))
    ).astype(np.float32)

    # cast + relayout to [chunk, group, row, slot, col]: fully-contiguous
    # runs per partition row for every device DMA
    ehi = (
        edges[:, hi]
        .astype(np.float16)
        .reshape(B, NHI // EG, EG, NCHUNK, P, H)
        .transpose(0, 3, 1, 4, 2, 5)
    )
    elo = (
        edges[:, lo]
        .astype(F8NP)
        .reshape(B, (E - NHI) // EG, EG, NCHUNK, P, H)
        .transpose(0, 3, 1, 4, 2, 5)
    )

    in_maps = []
    for b in range(B):
        in_maps.append(
            {
                "ehi": np.ascontiguousarray(ehi[b]),
                "elo": np.ascontiguousarray(elo[b]),
                "x": np.ascontiguousarray(x[b]),
                "vt": v_t,
                "vp": v_p,
                "widh": widh,
                "widl": widl,
                "wrep": wrep_host,
            }
        )

    nc = _get_program()
    res = run_bass_kernel_spmd(nc, in_maps, list(range(N_CORES)), trace=_trace)
    global LAST_RESULT
    LAST_RESULT = res

    out = np.stack([res.results[b]["out"] for b in range(B)])
    return out[:, None, :, :].astype(np.float32)


LAST_RESULT = None


# revision 29
# speedup vs baseline: 1.6390x; 1.1706x over previous
"""Trainium2 Bass kernel for nn_Affinity1d (gnn_message_passing).

Math (see original module): with w_e, w_t, w_p = split(Wcat),
    out[b, 0, i, j] = sum_e w_e[e] * edges[b, e, i, j]
                    + (w_t @ Wt @ x[b])[i]       # s_t, varies over rows
                    + (w_p @ Wp @ x[b])[j]       # s_p, varies over cols
`adj` only contributes its spatial size -> never shipped to the device.

Sharding: data-parallel over batch B=8 across the 8 NeuronCores (one
batch per core); the tiny folded weights are replicated.

Per-core device kernel:
  - s_t, s_p computed on PE from x (fp32, exact): v.T @ x as K=128x2
    accumulating matmuls, then a K=1 ones-matmul broadcasts s_p across
    partitions and a DRAM-roundtrip DMA transposes s_t into per-partition
    columns.
  - The dominant term is a 16-channel weighted reduction over 512 MB of
    edges. The host sorts channels by |w_e| and ships the 8 largest in
    fp16 and the 8 smallest in fp8e4m3 (error contribution scales with
    the channel's weight): 24 MB/core of HBM traffic instead of 64 MB
    fp32, at bf16-implementation-level accuracy (absmax rel err ~3e-3,
    resid_var ~5e-6). Host also relayouts both tensors to per-chunk
    blocks so every DMA reads fully-contiguous runs per partition row,
    streaming on both HWDGE rings.
  - The reduction is split across engines: 14 channels run as
    PSUM-accumulating matmuls with scaled-identity stationary weights
    (psum += w_e * I @ tile_e, channel-outer/half-inner so LDWEIGHTS
    hides under the paired matmul); one channel of each fp16 group
    reduces on the otherwise-idle vector engine (acc = e*w + acc chains
    seeded with the broadcast s_p, fp32 weights), keeping the PE well
    under the DMA pace.
  - Chunk 0's loads+matmuls are emitted before the s_t/s_p setup compute
    so the PE instruction stream is not head-blocked by the setup DMAs.
  - One DVE scalar_tensor_tensor pass per output half fuses
    out = psum + s_t[per-partition] + acc; the output is stored as fp16
    and upcast on host.

Measured on the 8 axon trn2 cores: 91-112 us HW exec (median ~103 us;
bimodal run-to-run variance tracks how the two NeuronCores sharing each
HBM stack interleave, ~380 vs ~330 GB/s effective). Fast-mode breakdown:
8.6 us NEFF preamble + ~74 us gap-free streaming + ~6 us tail chain +
~9 us Tile epilogue. L2 rel err 2.3e-3, absmax rel err 3.0e-3 (resid_var
5.4e-6) -- bf16-implementation-level accuracy. The fp16-only ancestor
(kernel_v6_backup.py) runs 108.5-127 us at L2 2.1e-4 if a tighter
accuracy gate is ever needed.
"""

import sys

if "/opt/trn_rl_repo" not in sys.path:
    sys.path.insert(0, "/opt/trn_rl_repo")

import numpy as np

from concourse import bacc, bass, mybir, tile
from concourse.bass_utils import run_bass_kernel_spmd

B, H, NIN, C, E = 8, 1024, 256, 128, 16
N_CORES = 8
P = 128          # partitions / rows per output chunk
NCHUNK = H // P  # 8 row-chunks per core
EG = 4           # edge channels per DMA group
FD = 512         # matmul free dim (one PSUM bank of fp32)
NHI = 8          # channels kept in fp16 (largest |w_e|); rest fp8e4m3

F32 = mybir.dt.float32
F16 = mybir.dt.float16
F8 = mybir.dt.float8e4
F8NP = mybir.dt.np(F8)

# slot layout after the host's |w_e| sort: slots 0-7 fp16 (groups 0-1),
# slots 8-15 fp8 (groups 2-3). One channel of each fp16 group reduces on
# the DVE with exact fp32 weights; the other 14 slots run on the PE.
DVE_SLOTS = [3, 7]
PE_SLOTS = [s for s in range(E) if s not in DVE_SLOTS]

_CACHED = None


def _build_program():
    nc = bacc.Bacc("TRN2", debug=False, num_devices=N_CORES)

    # host-relayouted: [chunk, group, row, slot_in_group, col] so each
    # (chunk, group) DMA reads fully-contiguous runs per partition row
    ehi_d = nc.dram_tensor(
        "ehi", [NCHUNK, NHI // EG, P, EG, H], F16, kind="ExternalInput"
    )
    elo_d = nc.dram_tensor(
        "elo", [NCHUNK, (E - NHI) // EG, P, EG, H], F8, kind="ExternalInput"
    )
    x_d = nc.dram_tensor("x", [NIN, H], F32, kind="ExternalInput")
    vt_d = nc.dram_tensor("vt", [NIN, 1], F32, kind="ExternalInput")
    vp_d = nc.dram_tensor("vp", [NIN, 1], F32, kind="ExternalInput")
    widh_d = nc.dram_tensor("widh", [P, NHI, P], F16, kind="ExternalInput")
    widl_d = nc.dram_tensor("widl", [P, E - NHI, P], F8, kind="ExternalInput")
    wrep_d = nc.dram_tensor("wrep", [P, len(DVE_SLOTS)], F32, kind="ExternalInput")
    out_d = nc.dram_tensor("out", [H, H], F16, kind="ExternalOutput")

    st_scratch = nc.dram_tensor("st_scratch", [1, H], F32)

    with tile.TileContext(nc) as tc:
        with (
            tc.tile_pool(name="const", bufs=1) as const,
            tc.tile_pool(name="setup_psum", bufs=1, space="PSUM") as spsum,
            tc.tile_pool(name="ehi", bufs=10) as ehpool,
            tc.tile_pool(name="elo", bufs=10) as elpool,
            tc.tile_pool(name="accs", bufs=6) as apool,
            tc.tile_pool(name="outs", bufs=3) as opool,
            tc.tile_pool(name="mpsum", bufs=3, space="PSUM") as mpsum,
        ):
            # ---- constant loads ----
            # weights head the sync HWDGE ring (no deps -> no FIFO stall);
            # x/vt/vp head the scalar ring so they land well before the
            # interleaved setup compute needs them.
            widh = const.tile([P, NHI, P], F16, tag="widh")
            widl = const.tile([P, E - NHI, P], F8, tag="widl")
            nc.sync.dma_start(widh[:], widh_d[:])
            nc.sync.dma_start(widl[:], widl_d[:])

            x0 = const.tile([P, H], F32, tag="x0")
            x1 = const.tile([P, H], F32, tag="x1")
            nc.scalar.dma_start(x0[:], x_d[0:P, :])
            nc.scalar.dma_start(x1[:], x_d[P : 2 * P, :])

            vt0 = const.tile([P, 1], F32, tag="vt0")
            vt1 = const.tile([P, 1], F32, tag="vt1")
            vp0 = const.tile([P, 1], F32, tag="vp0")
            vp1 = const.tile([P, 1], F32, tag="vp1")
            nc.scalar.dma_start(vt0[:], vt_d[0:P, :])
            nc.scalar.dma_start(vt1[:], vt_d[P : 2 * P, :])
            nc.scalar.dma_start(vp0[:], vp_d[0:P, :])
            nc.scalar.dma_start(vp1[:], vp_d[P : 2 * P, :])

            ones = const.tile([1, P], F32, tag="ones")
            nc.gpsimd.memset(ones[:], 1.0)

            wrep = const.tile([P, len(DVE_SLOTS)], F32, tag="wrep")
            nc.scalar.dma_start(wrep[:], wrep_d[:])

            st_cols = const.tile([P, NCHUNK], F32, tag="st_cols")
            sp_rep = const.tile([P, H], F32, tag="sp_rep")

            add = mybir.AluOpType.add
            mult = mybir.AluOpType.mult

            def slot_tile(etiles, s):
                if s < NHI:
                    return etiles[s // EG], s % EG
                return etiles[NHI // EG + (s - NHI) // EG], (s - NHI) % EG

            def emit_loads_mms(c):
                etiles = []
                ngroups = NHI // EG + (E - NHI) // EG
                for g in range(ngroups):
                    is_lo = g >= NHI // EG
                    pool = elpool if is_lo else ehpool
                    dt = F8 if is_lo else F16
                    src = elo_d[c, g - NHI // EG] if is_lo else ehi_d[c, g]
                    if c == NCHUNK - 1 and g == ngroups - 1:
                        # final transfer: split 4-ways so the tail matmuls
                        # start as soon as each slice lands
                        t = pool.tile([P, EG, H], dt, name="edgelast", tag="e")
                        # slices arrive in consumption order: after the last
                        # one lands only slot 15's two matmuls remain
                        for el in range(EG):
                            eng = nc.sync if el % 2 == 0 else nc.scalar
                            eng.dma_start(t[:, el : el + 1, :], src[:, el : el + 1, :])
                        etiles.append(t)
                        continue
                    t = pool.tile([P, EG, H], dt, name="edge", tag="e")
                    dma_eng = nc.sync if (c * ngroups + g) % 2 == 0 else nc.scalar
                    dma_eng.dma_start(t[:], src)
                    etiles.append(t)

                pss = [
                    mpsum.tile([P, FD], F32, name=f"ps{jh}", tag=f"ps{jh}")
                    for jh in range(2)
                ]
                # slot-outer / jh-inner: consecutive matmul pairs share the
                # stationary weights, so the next LDWEIGHTS hides under the
                # paired matmul. Groups for the two PSUM banks interleave.
                for si, s in enumerate(PE_SLOTS):
                    t, el = slot_tile(etiles, s)
                    lhsT = widh[:, s, :] if s < NHI else widl[:, s - NHI, :]
                    for jh in range(2):
                        sl = slice(jh * FD, (jh + 1) * FD)
                        nc.tensor.matmul(
                            pss[jh][:],
                            lhsT,
                            t[:, el, sl],
                            start=(si == 0),
                            stop=(si == len(PE_SLOTS) - 1),
                            skip_group_check=True,
                        )
                return etiles, pss

            def emit_dve_acc(c, etiles):
                # DVE reduces its (fp16) channels into an fp32 accumulator
                # seeded with the broadcast s_p term: acc = e*w + prev.
                prev = sp_rep
                for k, s in enumerate(DVE_SLOTS):
                    t, el = slot_tile(etiles, s)
                    acc = apool.tile([P, H], F32, name="acc", tag="acc")
                    nc.vector.scalar_tensor_tensor(
                        out=acc[:],
                        in0=t[:, el, :],
                        scalar=wrep[:, k : k + 1],
                        in1=prev[:],
                        op0=mult,
                        op1=add,
                    )
                    prev = acc
                return prev

            def emit_combine_store(c, pss, acc):
                rows = slice(c * P, (c + 1) * P)
                # Final chunk: split the store per half onto the (by now idle)
                # HWDGE rings so the kernel tail drains sooner.
                if c == NCHUNK - 1:
                    for jh, eng in ((0, nc.sync), (1, nc.scalar)):
                        sl = slice(jh * FD, (jh + 1) * FD)
                        oth = opool.tile([P, FD], F16, name=f"otl{jh}", tag=f"otl{jh}")
                        nc.vector.scalar_tensor_tensor(
                            out=oth[:],
                            in0=pss[jh][:],
                            scalar=st_cols[:, c : c + 1],
                            in1=acc[:, sl],
                            op0=add,
                            op1=add,
                        )
                        eng.dma_start(out_d[rows, sl], oth[:])
                else:
                    ot = opool.tile([P, H], F16, name="ot", tag="ot")
                    for jh in range(2):
                        sl = slice(jh * FD, (jh + 1) * FD)
                        nc.vector.scalar_tensor_tensor(
                            out=ot[:, sl],
                            in0=pss[jh][:],
                            scalar=st_cols[:, c : c + 1],
                            in1=acc[:, sl],
                            op0=add,
                            op1=add,
                        )
                    nc.gpsimd.dma_start(out_d[rows, :], ot[:])

            # Chunk 0's loads + matmuls are emitted FIRST so the PE starts
            # on the streaming reduction as soon as the weights + first
            # tile land. The s_t/s_p setup compute is interleaved after it;
            # only chunk 0's DVE work waits for the setup results, and by
            # then they are long done.
            etiles0, pss0 = emit_loads_mms(0)

            # s_t / s_p rows: (1, H) = v.T @ x, K=256 split into 2 matmuls
            st_row = const.tile([1, H], F32, tag="st_row")
            sp_row = const.tile([1, H], F32, tag="sp_row")
            for row, v0, v1 in ((st_row, vt0, vt1), (sp_row, vp0, vp1)):
                for jh in range(2):
                    ps = spsum.tile([1, FD], F32, name="sps", tag="sps")
                    sl = slice(jh * FD, (jh + 1) * FD)
                    nc.tensor.matmul(ps[:], v0[:], x0[:, sl], start=True, stop=False)
                    nc.tensor.matmul(ps[:], v1[:], x1[:, sl], start=False, stop=True)
                    nc.vector.tensor_copy(row[:, sl], ps[:])

            # s_t as per-partition columns: (P, NCHUNK), st_cols[p, c] = s_t[c*P+p]
            nc.gpsimd.dma_start(st_scratch[:], st_row[:])
            nc.gpsimd.dma_start(
                st_cols[:],
                st_scratch[:].rearrange("o (c p) -> (o p) c", p=P),
            )

            # s_p broadcast across partitions: (P, H)
            for jh in range(2):
                pb = spsum.tile([P, FD], F32, name="spb", tag="spb")
                sl = slice(jh * FD, (jh + 1) * FD)
                nc.tensor.matmul(pb[:], ones[:], sp_row[:, sl], start=True, stop=True)
                nc.vector.tensor_copy(sp_rep[:, sl], pb[:])

            # chunk 0's DVE chain must come after the setup copies in the
            # DVE stream (it consumes sp_rep/st_cols)
            acc0 = emit_dve_acc(0, etiles0)
            emit_combine_store(0, pss0, acc0)

            for c in range(1, NCHUNK):
                etiles, pss = emit_loads_mms(c)
                acc = emit_dve_acc(c, etiles)
                emit_combine_store(c, pss, acc)

    nc.compile()
    return nc


def _get_program():
    global _CACHED
    if _CACHED is None:
        _CACHED = _build_program()
    return _CACHED


def kernel(adj, edges, x, Wt, Wp, Wcat, _trace=False):
    del adj  # only its spatial size matters; unused numerically

    edges = np.asarray(edges, dtype=np.float32)
    x = np.asarray(x, dtype=np.float32)
    Wt = np.asarray(Wt, dtype=np.float32)
    Wp = np.asarray(Wp, dtype=np.float32)
    Wcat = np.asarray(Wcat, dtype=np.float32)

    # Fold the 1x1-conv weights: the theta/phi paths collapse to vectors.
    w_e = Wcat[:E]
    v_t = (Wcat[E : E + C] @ Wt).astype(np.float32).reshape(NIN, 1)
    v_p = (Wcat[E + C :] @ Wp).astype(np.float32).reshape(NIN, 1)

    # Sort channels by |w_e|: a channel's quantization error contributes
    # proportionally to its weight, so the 8 largest ship as fp16 and the
    # 8 smallest as fp8e4m3.
    order = np.argsort(-np.abs(w_e))
    hi, lo = order[:NHI], order[NHI:]

    eye = np.eye(P, dtype=np.float32)
    widh = (eye[:, None, :] * w_e[hi][None, :, None]).astype(np.float16)
    widl = (eye[:, None, :] * w_e[lo][None, :, None]).astype(F8NP)
    wrep_host = np.ascontiguousarray(
        np.broadcast_to(w_e[hi[DVE_SLOTS]], (P, len(DVE_SLOTS)))
    ).astype(np.float32)

    # cast + relayout to [chunk, group, row, slot, col]: fully-contiguous
    # runs per partition row for every device DMA
    ehi = (
        edges[:, hi]
        .astype(np.float16)
        .reshape(B, NHI // EG, EG, NCHUNK, P, H)
        .transpose(0, 3, 1, 4, 2, 5)
    )
    elo = (
        edges[:, lo]
        .astype(F8NP)
        .reshape(B, (E - NHI) // EG, EG, NCHUNK, P, H)
        .transpose(0, 3, 1, 4, 2, 5)
    )

    in_maps = []
    for b in range(B):
        in_maps.append(
            {
                "ehi": np.ascontiguousarray(ehi[b]),
                "elo": np.ascontiguousarray(elo[b]),
                "x": np.ascontiguousarray(x[b]),
                "vt": v_t,
                "vp": v_p,
                "widh": widh,
                "widl": widl,
                "wrep": wrep_host,
            }
        )

    nc = _get_program()
    res = run_bass_kernel_spmd(nc, in_maps, list(range(N_CORES)), trace=_trace)
    global LAST_RESULT
    LAST_RESULT = res

    out = np.stack([res.results[b]["out"] for b in range(B)])
    return out[:, None, :, :].astype(np.float32)


LAST_RESULT = None
